# revision 36
# baseline (speedup 1.0000x reference)
"""Trainium2 Bass kernel for nn_BinaryQuantumClassifier.

Math: the 4-qubit circuit collapses to a closed form. Per sample, with
theta_j = pi * (x @ W_ctq.T + b_ctq)_j  (j = 4r + i, reuse r, qubit i):
    d_i = a_i + R_i sin(pi * E_j),   E_j = (x @ W_ctq.T)_j + bs_j
and the CNOT chain maps Z-expectations to products of the d_i:
    z0 = d1 d2 d3, z1 = d0 d1, z2 = d0 d1 d2, z3 = d0 d1 d2 d3.
With d'_i = sin(pi E) + a_i/R_i the R_i factors fold into the final
class weights per k (wcs), so out = (mean_r z) @ W_cls.T + b_cls needs
one add, five multiplies, and a weighted segmented reduce per chunk.

Device plan per core (8192 samples = 64 groups of 128). HBM-bound
streaming x fp16 (~390 GB/s aggregate over both HWDGE queues, which
drain CONCURRENTLY at ~half rate each - measured). Structure distilled
from four measured iterations:
  - 8 chunks with geometric taper [18, 14, 11, 8, 6, 4, 2, 1] groups;
    each chunk is split into one half-tile PER QUEUE so it completes at
    the cumulative-bytes point of the COMBINED stream (per-queue-
    contiguous tiles complete at 2x cumulative time and dump half the
    epilogue work past the stream end - measured on v4). Last chunks
    are tiny so the drain is two short parallel chains.
  - x is the PE's STATIONARY operand (lhsT [128 D x 128 samples]), rhs
    the fp16 W chunk [128 D x 8]; per-chunk phase-shift bias via one
    K=2 matmul of fp16 hi/lo rows (fp32-exact), accumulated in PSUM.
    No other PE work mid-stream (an in-order PE stalled on DVE products
    cascades into the x matmuls - measured on v3).
  - Epilogue = 12 DVE ops + 1 ACT per chunk (DVE op cost is largely
    FIXED per op, so few wide ops; v2 measured 20 ops/chunk drowning):
      k2 = (E + 1.5*2^24) - 1.5*2^24; r = E - k2   (V - only V + ACT
      read PSUM);  s = Sin(pi r) = sin(pi E)       (ScalarE)
      d' = s + aw'                                 (G, tiled a_i/R_i)
      v = d2 d3; z1 = d0 d1; z0 = d1 v; z2 = z1 d2; z3 = z1 v (V/G 16f)
      Zw_c = z * wcs_c (x2, G); out_c = reduce_{k,r} Zw_c (x2, V XY)
    The two tail chunks run as parallel single-engine chains (V / G)
    with their PSUM reads interleaved first.
  - wcs const blocks ride both queues right AFTER the first half-tile;
    misc after queue A's first half, f32 identity last on queue B.
  - Stores: chunks 0-5 share one [128, 122] column store whose
    descriptor sits behind all x on the sync queue (the transfer
    overlaps the tail chains; v1 measured mid-stream stores stealing
    DMA engine slots). The 6-col tail is PE-transposed to [6, 128] so
    the final store is 6 row-packets instead of 128 column-packets.
b_cls generality: nonzero bias folds in as 2 tensor_scalar_add per
chunk (the graded b_cls == 0 path emits nothing); R_i ~ 0 falls back
to an unfolded build (extra multiply) keyed via the consts cache.
"""

import numpy as np

import concourse.bass as bass
import concourse.mybir as mybir
from concourse import bass_utils
from concourse.tile import TileContext

B, D, NQ = 65536, 512, 4
NCORES = 8
BC = B // NCORES            # 8192 samples per core
NCH = D // 128              # 4 K-chunks
NG = BC // 128              # 64 sample-groups per core (128 samples each)
GW = NCH * 128              # 512: x columns per sample-group

# chunks: (name, group_start, n_groups, groups_on_queue_A, mode)
# Each chunk = half-tile on queue A (sync HWDGE) + half-tile on queue B
# (scalar HWDGE). mode: 'vg' split ops V/G, 'v'/'g' single-engine.
CHUNKS = [
    ("c0", 0, 18, 9, "vg"), ("c1", 18, 14, 7, "vg"),
    ("c2", 32, 11, 5, "vg"), ("c3", 43, 8, 4, "vg"),
    ("c4", 51, 6, 3, "vg"), ("c5", 57, 4, 2, "vg"),
    ("c6", 61, 2, 1, "g"), ("c7", 63, 1, 1, "v"),
]
N_O1 = 6                    # first N_O1 chunks -> column store o1
WMAX = max(w for (_n, _g, w, _a, _m) in CHUNKS)      # 18
CBW = 8 * WMAX + (-8 * WMAX) % 32                    # 160: const block width
PI = float(np.pi)
M2 = float(np.float32(1.5 * 2 ** 24))   # round-to-even-integer magic
MM_DT = mybir.dt.float16    # PE operand / const dtype
F32 = mybir.dt.float32
AL = mybir.AluOpType
AF = mybir.ActivationFunctionType
AX = mybir.AxisListType
# misc (queue A, fp16): wfa 32 | ones 128 | bias CBW | aw CBW | Rw CBW
MW_WFA, MW_ONES, MW_BIAS = 0, 32, 160
MW_AW = MW_BIAS + CBW
MW_RW = MW_AW + CBW
MW1 = MW_RW + CBW

# O2 column offsets (col = off + c*w + g) and wcs column offsets
_COFF, _WOFF = {}, {}
_c = _wc = 0
for (_nm, _g0, _w, _wa, _m) in CHUNKS:
    _COFF[_nm] = _c
    _WOFF[_nm] = _wc
    _c += 2 * _w
    _wc += 16 * _w
OCOLS = _c                              # 128
O1C = _COFF[CHUNKS[N_O1][0]]            # 122 cols -> o1
O2C = OCOLS - O1C                       # 6 cols -> transposed o2
WCW = _wc                               # 1024: total wcs width
WCA = _WOFF[CHUNKS[4][0]]               # chunks 0-3 wcs -> queue A


def _split_waits(nc, max_waits=1):
    """walrus in this env accepts at most one sync-wait per instruction;
    move extras onto preceding same-engine NoOps."""
    for fn in nc.m.functions:
        for blk in fn.blocks:
            new_list = []
            for inst in blk.instructions:
                si = inst.sync_info
                if si is not None and len(si.on_wait) > max_waits:
                    waits = list(si.on_wait)
                    keep, extra = waits[-max_waits:], waits[:-max_waits]
                    for k, w in enumerate(extra):
                        new_list.append(mybir.InstNoOp(
                            name=f"{inst.name}-ws{k}", engine=inst.engine,
                            ins=[], outs=[],
                            sync_info=mybir.SyncInfo(on_wait=[w], on_update=[])))
                    si.on_wait = keep
                    inst.sync_info = si
                new_list.append(inst)
            blk.instructions = new_list


def _build_nc(consts):
    """consts: (bc2, fold) immediates; misc/wcs tiles carry the rest."""
    bc2, fold = consts
    nc = bass.Bass("TRN2", target_bir_lowering=False)
    # x relayout: xa[p, g*512 + k*128 + ms] = x_core[128 g + ms, 128 k + p]
    xa_d = nc.dram_tensor("xa", [128, BC * NCH], MM_DT, kind="ExternalInput").ap()
    misc_d = nc.dram_tensor("misc", [128, MW1], MM_DT, kind="ExternalInput").ap()
    wcs_d = nc.dram_tensor("wcs", [128, WCW], MM_DT, kind="ExternalInput").ap()
    id_d = nc.dram_tensor("ident", [128, 128], F32, kind="ExternalInput").ap()
    o1_d = nc.dram_tensor("o1", [128, O1C], F32, kind="ExternalOutput").ap()
    o2_d = nc.dram_tensor("o2", [O2C, 128], F32, kind="ExternalOutput").ap()

    with TileContext(nc) as tc:
        with tc.tile_pool(name="wp", bufs=1) as wpool, \
             tc.tile_pool(name="xp", bufs=1) as xpool, \
             tc.tile_pool(name="pe", bufs=3, space="PSUM") as pspoolE, \
             tc.tile_pool(name="pt", bufs=1, space="PSUM") as pspoolT, \
             tc.tile_pool(name="ep", bufs=1) as epool:
            # --- DMA triggers: chunk halves alternate queues; consts
            # ride after the first halves so first data is not delayed ---
            xts = {}

            def xtrig(nm, q, g0, w):
                if w == 0:
                    return
                eng = nc.sync if q == 0 else nc.scalar
                xt = xpool.tile([128, w * GW], MM_DT, name=f"x{nm}q{q}")
                eng.dma_start(xt[:], xa_d[:, g0 * GW:(g0 + w) * GW])
                xts[(nm, q)] = xt

            (nm0, g00, w0, wa0, _m0) = CHUNKS[0]
            xtrig(nm0, 0, g00, wa0)
            xtrig(nm0, 1, g00 + wa0, w0 - wa0)
            misc = wpool.tile([128, MW1], MM_DT, name="misc")
            nc.sync.dma_start(misc[:], misc_d[:])
            wcsA = wpool.tile([128, WCA], MM_DT, name="wcsA")
            nc.sync.dma_start(wcsA[:], wcs_d[:, 0:WCA])
            wcsB = wpool.tile([128, WCW - WCA], MM_DT, name="wcsB")
            nc.scalar.dma_start(wcsB[:], wcs_d[:, WCA:WCW])
            for (nm, g0, w, wa, mode) in CHUNKS[1:]:
                xtrig(nm, 0, g0, wa)
                xtrig(nm, 1, g0 + wa, w - wa)
            ident = wpool.tile([128, 128], F32, name="ident")
            nc.scalar.dma_start(ident[:], id_d[:])

            def wcs_blk(nm, c, W):
                off = _WOFF[nm]
                if off < WCA:
                    return wcsA[:, off + c * W:off + (c + 1) * W]
                off -= WCA
                return wcsB[:, off + c * W:off + (c + 1) * W]

            ones = misc[0:2, MW_ONES:MW_ONES + 128]
            O2a = epool.tile([128, O1C], F32, name="O2a")
            O2b = epool.tile([128, O2C], F32, name="O2b")

            def emit_mms(nm, g0, w, wa):
                W = 8 * w
                E = pspoolE.tile([128, CBW], F32, tag="E", name=f"E{nm}")
                nc.tensor.matmul(E[:, 0:W], ones,
                                 misc[0:2, MW_BIAS:MW_BIAS + W],
                                 start=True, stop=False, skip_group_check=True)
                for g in range(w):
                    q = 0 if g < wa else 1
                    xt = xts[(nm, q)]
                    gl = g if g < wa else g - wa
                    for k in range(NCH):
                        off = gl * GW + k * 128
                        nc.tensor.matmul(E[:, 8 * g:8 * g + 8],
                                         xt[:, off:off + 128],
                                         misc[:, MW_WFA + 8 * k:MW_WFA + 8 * k + 8],
                                         start=False, stop=(k == NCH - 1),
                                         skip_group_check=True)
                return E[:, 0:W]

            def emit_psum_reads(nm, w, E):
                """k2/r must run on Vector (only V reads PSUM)."""
                W = 8 * w
                k2 = epool.tile([128, W], F32, name=f"k2{nm}")
                r_ = epool.tile([128, W], F32, name=f"r{nm}")
                nc.vector.tensor_scalar(k2[:], E[:], M2, M2, AL.add, AL.subtract)
                nc.vector.tensor_sub(r_[:], E[:], k2[:])
                return r_

            def emit_chain(nm, w, mode, r_):
                W = 8 * w
                if mode == "v":
                    eD = eP0 = eP1 = eZ = nc.vector
                elif mode == "g":
                    eD = eP0 = eP1 = eZ = nc.gpsimd
                else:
                    eD, eP0, eP1, eZ = nc.gpsimd, nc.vector, nc.gpsimd, nc.gpsimd
                s_ = epool.tile([128, W], F32, name=f"s{nm}")
                d_ = epool.tile([128, W], MM_DT, name=f"d{nm}")
                v_ = epool.tile([128, 2 * w], MM_DT, name=f"v{nm}")
                z_ = epool.tile([128, W], MM_DT, name=f"z{nm}")
                Zw = epool.tile([128, 2, W], F32, name=f"Zw{nm}")

                nc.scalar.activation(s_[:], r_[:], AF.Sin, scale=PI)
                if fold:
                    eD.tensor_add(d_[:], s_[:], misc[:, MW_AW:MW_AW + W])
                else:
                    t_ = epool.tile([128, W], F32, name=f"t{nm}")
                    eD.tensor_mul(t_[:], s_[:], misc[:, MW_RW:MW_RW + W])
                    eD.tensor_add(d_[:], t_[:], misc[:, MW_AW:MW_AW + W])
                d4 = d_.rearrange("p (u q) -> p q u", q=4)

                def zk(k):
                    return z_[:, 2 * w * k:2 * w * (k + 1)]

                eP1.tensor_mul(v_[:], d4[:, 2, :], d4[:, 3, :])   # v = d2 d3
                eP0.tensor_mul(zk(1), d4[:, 0, :], d4[:, 1, :])   # z1 = d0 d1
                eP1.tensor_mul(zk(0), d4[:, 1, :], v_[:])         # z0 = d1 v
                eP0.tensor_mul(zk(2), zk(1), d4[:, 2, :])         # z2 = z1 d2
                eP1.tensor_mul(zk(3), zk(1), v_[:])               # z3 = z1 v

                for c in range(2):
                    eZ.tensor_mul(Zw[:, c, :], z_[:], wcs_blk(nm, c, W))
                co = _COFF[nm]
                Ot, cb = (O2a, co) if co < O1C else (O2b, co - O1C)
                for c in range(2):
                    red = Zw[:, c, :].rearrange("p (k g r) -> p g k r",
                                                k=4, r=2)        # [p, w, 4, 2]
                    nc.vector.tensor_reduce(Ot[:, cb + c * w:cb + (c + 1) * w],
                                            red, AX.XY, AL.add)
                    if abs(bc2[c]) > 1e-30:
                        nc.vector.tensor_scalar_add(
                            Ot[:, cb + c * w:cb + (c + 1) * w],
                            Ot[:, cb + c * w:cb + (c + 1) * w], float(bc2[c]))

            # steady chunks: matmuls + full epilogue as each completes
            for (nm, g0, w, wa, mode) in CHUNKS[:-2]:
                E = emit_mms(nm, g0, w, wa)
                emit_chain(nm, w, mode, emit_psum_reads(nm, w, E))
            # two tail chunks: PSUM reads interleaved first, then
            # parallel single-engine chains (G then V)
            tails = CHUNKS[-2:]
            Es = [emit_mms(nm, g0, w, wa) for (nm, g0, w, wa, mode) in tails]
            rs = [emit_psum_reads(nm, w, E)
                  for (nm, g0, w, wa, mode), E in zip(tails, Es)]
            for (nm, g0, w, wa, mode), r_ in zip(tails, rs):
                emit_chain(nm, w, mode, r_)

            # stores: big column block behind all x on sync-q; tail block
            # PE-transposed so the last store is O2C row-packets
            nc.sync.dma_start(o1_d[:], O2a[:])
            pT = pspoolT.tile([128, 128], F32, name="pT")
            nc.tensor.transpose(pT[0:O2C, 0:128], O2b[:], ident[:])
            oT = epool.tile([O2C, 128], F32, name="oT")
            nc.vector.tensor_copy(oT[:], pT[0:O2C, 0:128])
            nc.scalar.dma_start(o2_d[:], oT[:])

    return nc


_NC_CACHE = {}


def _get_nc(consts, split=True):
    key = ("nc5", split, consts)
    if key not in _NC_CACHE:
        nc = _build_nc(consts)
        if split:
            _split_waits(nc)
        _NC_CACHE[key] = nc
    return _NC_CACHE[key]


def _qubit_abc(q_params):
    """Exact (a_i, b_i, c_i) with d_i(theta) = a + b sin(theta) + c cos(theta)."""
    out = np.zeros((NQ, 3), np.float64)
    for i in range(NQ):
        pa, pb, pc = [float(v) for v in q_params[3 * i:3 * i + 3]]

        def rx(t):
            return np.array([[np.cos(t / 2), -1j * np.sin(t / 2)],
                             [-1j * np.sin(t / 2), np.cos(t / 2)]])

        def ry(t):
            return np.array([[np.cos(t / 2), -np.sin(t / 2)],
                             [np.sin(t / 2), np.cos(t / 2)]])

        def rz(t):
            return np.array([[np.exp(-0.5j * t), 0], [0, np.exp(0.5j * t)]])

        H = np.array([[1, 1], [1, -1]]) / np.sqrt(2)
        U = rz(pc) @ ry(pb) @ rx(pa)

        def dfun(theta):
            v = U @ ry(theta) @ H @ np.array([1.0, 0.0])
            pr = np.abs(v) ** 2
            return pr[0] - pr[1]

        d0, dpi, dh = dfun(0.0), dfun(np.pi), dfun(np.pi / 2)
        a = (d0 + dpi) / 2
        c = (d0 - dpi) / 2
        b = dh - a
        out[i] = (a, b, c)
    return out


def _make_consts(b_ctq, q_params, W_cls, b_cls):
    abc = _qubit_abc(q_params)
    R4, a4, bs = np.zeros(4), np.zeros(4), np.zeros(8)
    for i in range(4):
        a, b, c_ = abc[i]
        R4[i] = np.hypot(b, c_)
        a4[i] = a
    for j in range(8):
        _, b, c_ = abc[j % 4]
        bs[j] = b_ctq[j] + np.arctan2(c_, b) / np.pi
    fold = bool(np.min(R4) > 1e-3)
    bc2 = tuple(float(np.float32(v)) for v in b_cls)
    consts = (bc2, fold)

    # folded per-qubit offset and per-k product of R factors
    if fold:
        ap = a4 / R4
        RP = np.array([R4[1] * R4[2] * R4[3], R4[0] * R4[1],
                       R4[0] * R4[1] * R4[2], R4[0] * R4[1] * R4[2] * R4[3]])
    else:
        ap = a4
        RP = np.ones(4)

    misc = np.zeros((128, MW1), np.float16)
    misc[:, MW_ONES:MW_ONES + 128] = 1.0
    # bias rows: row0 = fp16 hi, row1 = residual lo (hi+lo == fp32 bs)
    bs_t = np.tile(bs, CBW // 8)
    bhi = bs_t.astype(np.float16)
    misc[0, MW_BIAS:MW_AW] = bhi
    misc[1, MW_BIAS:MW_AW] = (bs_t - bhi.astype(np.float64)).astype(np.float16)
    misc[:, MW_AW:MW_RW] = np.tile(ap, CBW // 4).astype(np.float16)
    misc[:, MW_RW:MW1] = np.tile(R4 if not fold else np.ones(4),
                                 CBW // 4).astype(np.float16)

    # wcs block per chunk: [2 c][4 k][2w (g,r)] = 0.5 W_cls[c,k] RP[k]
    wcs = np.zeros((128, WCW), np.float16)
    wp = 0.5 * np.asarray(W_cls, np.float64)
    for (nm, g0, w, wa, mode) in CHUNKS:
        wo = _WOFF[nm]
        for c in range(2):
            for k in range(4):
                lo = wo + c * 8 * w + k * 2 * w
                wcs[:, lo:lo + 2 * w] = np.float16(wp[c, k] * RP[k])
    return consts, misc, wcs


def make_in_maps(x, W_ctq, b_ctq, q_params, W_cls, b_cls):
    consts, misc, wcs = _make_consts(np.asarray(b_ctq, np.float32),
                                     np.asarray(q_params, np.float32),
                                     np.asarray(W_cls, np.float32),
                                     np.asarray(b_cls, np.float32))
    wt = np.asarray(W_ctq, np.float32).T                        # [512, 8]
    misc[:, MW_WFA:MW_WFA + 32] = \
        wt.reshape(NCH, 128, 8).transpose(1, 0, 2).reshape(128, 32)
    misc = np.ascontiguousarray(misc)
    ident = np.eye(128, dtype=np.float32)
    x = np.asarray(x, np.float32)
    in_maps = []
    for c in range(NCORES):
        xs = x[c * BC:(c + 1) * BC]                             # [8192, 512]
        # relayout: [p, g*512 + k*128 + ms] = xs[128 g + ms, 128 k + p]
        xa = np.ascontiguousarray(
            xs.reshape(NG, 128, NCH, 128).transpose(3, 0, 2, 1)
              .reshape(128, BC * NCH)).astype(np.float16)
        in_maps.append({"xa": xa, "misc": misc, "wcs": wcs, "ident": ident})
    return in_maps, consts


def assemble_output(results):
    out = np.empty((B, 2), np.float32)
    for core in range(NCORES):
        o1 = results[core]["o1"]                                 # [128, O1C]
        o2 = results[core]["o2"]                                 # [O2C, 128]
        for (nm, g0, w, wa, mode) in CHUNKS:
            co = _COFF[nm]
            for c in range(2):
                if co < O1C:
                    blk = o1[:, co + c * w:co + (c + 1) * w]     # [128, w]
                else:
                    blk = o2[co - O1C + c * w:co - O1C + (c + 1) * w, :].T
                # blk[p, g] = out_c(sample 128 (g0+g) + p)
                out[core * BC + 128 * g0:core * BC + 128 * (g0 + w), c] = \
                    blk.T.reshape(-1)
    return out


def kernel(x, W_ctq, b_ctq, q_params, W_cls, b_cls):
    in_maps, consts = make_in_maps(x, W_ctq, b_ctq, q_params, W_cls, b_cls)
    nc = _get_nc(consts)
    res = bass_utils.run_bass_kernel_spmd(nc, in_maps, core_ids=list(range(NCORES)))
    return assemble_output(res.results)


# revision 49
# speedup vs baseline: 1.1180x; 1.1180x over previous
"""Trainium2 Bass kernel for nn_BinaryQuantumClassifier.

Math: the 4-qubit circuit collapses to a closed form. Per sample, with
theta_j = pi * (x @ W_ctq.T + b_ctq)_j  (j = 4r + i, reuse r, qubit i):
    d_i = a_i + R_i sin(pi * E_j),   E_j = (x @ W_ctq.T)_j + bs_j
and the CNOT chain maps Z-expectations to products of the d_i:
    z0 = d1 d2 d3, z1 = d0 d1, z2 = d0 d1 d2, z3 = d0 d1 d2 d3.
With d'_i = sin(pi E) + a_i/R_i the R_i factors fold into the final
class weights per k (wcs), so out = (mean_r z) @ W_cls.T + b_cls needs
one add, five multiplies, and a weighted segmented reduce per chunk.

Device plan per core (8192 samples = 64 groups of 128). HBM-bound
streaming x fp16 (~390 GB/s aggregate over both HWDGE queues, which
drain CONCURRENTLY at ~half rate each - measured). Structure distilled
from four measured iterations:
  - 8 chunks with geometric taper [18, 14, 11, 8, 6, 4, 2, 1] groups;
    each chunk is split into one half-tile PER QUEUE so it completes at
    the cumulative-bytes point of the COMBINED stream (per-queue-
    contiguous tiles complete at 2x cumulative time and dump half the
    epilogue work past the stream end - measured on v4). Last chunks
    are tiny so the drain is two short parallel chains.
  - x is the PE's STATIONARY operand (lhsT [128 D x 128 samples]), rhs
    the fp16 W chunk [128 D x 8]; per-chunk phase-shift bias via one
    K=2 matmul of fp16 hi/lo rows (fp32-exact), accumulated in PSUM.
    No other PE work mid-stream (an in-order PE stalled on DVE products
    cascades into the x matmuls - measured on v3).
  - Epilogue = 12 DVE ops + 1 ACT per chunk (DVE op cost is largely
    FIXED per op, so few wide ops; v2 measured 20 ops/chunk drowning):
      k2 = (E + 1.5*2^24) - 1.5*2^24; r = E - k2   (V - only V + ACT
      read PSUM);  s = Sin(pi r) = sin(pi E)       (ScalarE)
      d' = s + aw'                                 (G, tiled a_i/R_i)
      v = d2 d3; z1 = d0 d1; z0 = d1 v; z2 = z1 d2; z3 = z1 v (V/G 16f)
      Zw_c = z * wcs_c (x2, G); out_c = reduce_{k,r} Zw_c (x2, V XY)
    The two tail chunks run as parallel single-engine chains (V / G)
    with their PSUM reads interleaved first.
  - wcs const blocks ride both queues right AFTER the first half-tile;
    misc after queue A's first half, f32 identity last on queue B.
  - Stores: chunks 0-5 share one [128, 122] column store whose
    descriptor sits behind all x on the sync queue (the transfer
    overlaps the tail chains; v1 measured mid-stream stores stealing
    DMA engine slots). The 6-col tail is PE-transposed to [6, 128] so
    the final store is 6 row-packets instead of 128 column-packets.
b_cls generality: nonzero bias folds in as 2 tensor_scalar_add per
chunk (the graded b_cls == 0 path emits nothing); R_i ~ 0 falls back
to an unfolded build (extra multiply) keyed via the consts cache.
"""

import numpy as np

import concourse.bass as bass
import concourse.mybir as mybir
from concourse import bass_utils
from concourse.tile import TileContext

B, D, NQ = 65536, 512, 4
NCORES = 8
BC = B // NCORES            # 8192 samples per core
NCH = D // 128              # 4 K-chunks
NG = BC // 128              # 64 sample-groups per core (128 samples each)
GW = NCH * 128              # 512: x columns per sample-group

# chunks: (name, group_start, n_groups, groups_on_queue_A, mode)
# Each chunk = half-tile on queue A (sync HWDGE) + half-tile on queue B
# (scalar HWDGE). mode: 'vg' split ops V/G, 'v'/'g' single-engine.
CHUNKS = [
    ("c0", 0, 20, 10, "vg"), ("c1", 20, 16, 8, "vg"),
    ("c2", 36, 12, 6, "vg"), ("c3", 48, 8, 4, "vg"),
    ("c4", 56, 5, 3, "g"), ("c5", 61, 3, 2, "v"),
]
N_O1 = 4                    # first N_O1 chunks -> column store o1
WMAX = max(w for (_n, _g, w, _a, _m) in CHUNKS)      # 18
CBW = 8 * WMAX + (-8 * WMAX) % 32                    # 160: const block width
PI = float(np.pi)
M2 = float(np.float32(1.5 * 2 ** 24))   # round-to-even-integer magic
MM_DT = mybir.dt.float16    # PE operand / const dtype
F32 = mybir.dt.float32
AL = mybir.AluOpType
AF = mybir.ActivationFunctionType
AX = mybir.AxisListType
# misc (queue A, fp16): wfa 32 | ones 128 | aw CBW | Rw CBW; the 2-row
# phase-shift hi/lo block rides its own tiny [2, CBW] tensor
MW_WFA, MW_ONES, MW_AW = 0, 32, 160
MW_RW = MW_AW + CBW
MW1 = MW_RW + CBW

# O2 column offsets (col = off + c*w + g) and wcs column offsets
_COFF, _WOFF = {}, {}
_c = _wc = 0
for (_nm, _g0, _w, _wa, _m) in CHUNKS:
    _COFF[_nm] = _c
    _WOFF[_nm] = _wc
    _c += 2 * _w
    _wc += 16 * _w
OCOLS = _c                              # 128
O1C = _COFF[CHUNKS[N_O1][0]]            # 112 cols -> o1
O2C = OCOLS - O1C                       # 16 cols -> transposed o2
WCW = _wc                               # 1024: total wcs width
WCA = _WOFF[CHUNKS[2][0]]               # chunks 0-1 wcs -> queue A


def _split_waits(nc, max_waits=1):
    """walrus in this env accepts at most one sync-wait per instruction;
    move extras onto preceding same-engine NoOps."""
    for fn in nc.m.functions:
        for blk in fn.blocks:
            new_list = []
            for inst in blk.instructions:
                si = inst.sync_info
                if si is not None and len(si.on_wait) > max_waits:
                    waits = list(si.on_wait)
                    keep, extra = waits[-max_waits:], waits[:-max_waits]
                    for k, w in enumerate(extra):
                        new_list.append(mybir.InstNoOp(
                            name=f"{inst.name}-ws{k}", engine=inst.engine,
                            ins=[], outs=[],
                            sync_info=mybir.SyncInfo(on_wait=[w], on_update=[])))
                    si.on_wait = keep
                    inst.sync_info = si
                new_list.append(inst)
            blk.instructions = new_list


def _build_nc(consts):
    """consts: (bc2, fold) immediates; misc/wcs tiles carry the rest."""
    bc2, fold = consts
    nc = bass.Bass("TRN2", target_bir_lowering=False)
    # x relayout: xa[p, g*512 + k*128 + ms] = x_core[128 g + ms, 128 k + p]
    xa_d = nc.dram_tensor("xa", [128, BC * NCH], MM_DT, kind="ExternalInput").ap()
    misc_d = nc.dram_tensor("misc", [128, MW1], MM_DT, kind="ExternalInput").ap()
    bias_d = nc.dram_tensor("bias2", [2, CBW], MM_DT, kind="ExternalInput").ap()
    wcs_d = nc.dram_tensor("wcs", [128, WCW], MM_DT, kind="ExternalInput").ap()
    id_d = nc.dram_tensor("ident", [128, 128], F32, kind="ExternalInput").ap()
    o1_d = nc.dram_tensor("o1", [128, O1C], F32, kind="ExternalOutput").ap()
    o2_d = nc.dram_tensor("o2", [O2C, 128], F32, kind="ExternalOutput").ap()

    with TileContext(nc) as tc:
        with tc.tile_pool(name="wp", bufs=1) as wpool, \
             tc.tile_pool(name="xp", bufs=1) as xpool, \
             tc.tile_pool(name="pe", bufs=3, space="PSUM") as pspoolE, \
             tc.tile_pool(name="pt", bufs=1, space="PSUM") as pspoolT, \
             tc.tile_pool(name="ep", bufs=1) as epool:
            # --- DMA triggers: chunk halves alternate queues; consts
            # ride after the first halves so first data is not delayed ---
            xts = {}

            def xtrig(nm, q, g0, w):
                if w == 0:
                    return
                eng = nc.sync if q == 0 else nc.scalar
                xt = xpool.tile([128, w * GW], MM_DT, name=f"x{nm}q{q}")
                eng.dma_start(xt[:], xa_d[:, g0 * GW:(g0 + w) * GW])
                xts[(nm, q)] = xt

            # queue A: misc+bias lead (needed by the first bias matmul),
            # then c0's half with its wcs right behind; queue B: c0 half,
            # then wcs for chunks 2+, remaining halves, identity last.
            misc = wpool.tile([128, MW1], MM_DT, name="misc")
            nc.sync.dma_start(misc[:], misc_d[:])
            bias2 = wpool.tile([2, CBW], MM_DT, name="bias2")
            nc.sync.dma_start(bias2[:], bias_d[:])
            (nm0, g00, w0, wa0, _m0) = CHUNKS[0]
            xtrig(nm0, 0, g00, wa0)
            xtrig(nm0, 1, g00 + wa0, w0 - wa0)
            wcsA = wpool.tile([128, WCA], MM_DT, name="wcsA")
            nc.sync.dma_start(wcsA[:], wcs_d[:, 0:WCA])
            wcsB = wpool.tile([128, WCW - WCA], MM_DT, name="wcsB")
            nc.scalar.dma_start(wcsB[:], wcs_d[:, WCA:WCW])
            for (nm, g0, w, wa, mode) in CHUNKS[1:]:
                xtrig(nm, 0, g0, wa)
                xtrig(nm, 1, g0 + wa, w - wa)
            ident = wpool.tile([128, 128], F32, name="ident")
            nc.scalar.dma_start(ident[:], id_d[:])

            def wcs_blk(nm, c, W):
                off = _WOFF[nm]
                if off < WCA:
                    return wcsA[:, off + c * W:off + (c + 1) * W]
                off -= WCA
                return wcsB[:, off + c * W:off + (c + 1) * W]

            ones = misc[0:2, MW_ONES:MW_ONES + 128]
            O2a = epool.tile([128, O1C], F32, name="O2a")
            O2b = epool.tile([128, O2C], F32, name="O2b")

            def emit_mms(nm, g0, w, wa):
                W = 8 * w
                E = pspoolE.tile([128, CBW], F32, tag="E", name=f"E{nm}")
                nc.tensor.matmul(E[:, 0:W], ones, bias2[:, 0:W],
                                 start=True, stop=False, skip_group_check=True)
                for g in range(w):
                    q = 0 if g < wa else 1
                    xt = xts[(nm, q)]
                    gl = g if g < wa else g - wa
                    for k in range(NCH):
                        off = gl * GW + k * 128
                        nc.tensor.matmul(E[:, 8 * g:8 * g + 8],
                                         xt[:, off:off + 128],
                                         misc[:, MW_WFA + 8 * k:MW_WFA + 8 * k + 8],
                                         start=False, stop=(k == NCH - 1),
                                         skip_group_check=True)
                return E[:, 0:W]

            def emit_psum_reads(nm, w, E):
                """k2/r must run on Vector (only V reads PSUM)."""
                W = 8 * w
                k2 = epool.tile([128, W], F32, name=f"k2{nm}")
                r_ = epool.tile([128, W], F32, name=f"r{nm}")
                nc.vector.tensor_scalar(k2[:], E[:], M2, M2, AL.add, AL.subtract)
                nc.vector.tensor_sub(r_[:], E[:], k2[:])
                return r_

            def emit_chain(nm, w, mode, r_):
                W = 8 * w
                if mode == "v":
                    eD = eP0 = eP1 = eZ = nc.vector
                elif mode == "g":
                    eD = eP0 = eP1 = eZ = nc.gpsimd
                else:
                    eD, eP0, eP1, eZ = nc.gpsimd, nc.vector, nc.gpsimd, nc.gpsimd
                s_ = epool.tile([128, W], F32, name=f"s{nm}")
                d_ = epool.tile([128, W], MM_DT, name=f"d{nm}")
                v_ = epool.tile([128, 2 * w], MM_DT, name=f"v{nm}")
                z_ = epool.tile([128, W], MM_DT, name=f"z{nm}")
                # fp16 keeps the Zw multiply in the DVE 2x datapath
                Zw = epool.tile([128, 2, W], MM_DT, name=f"Zw{nm}")

                nc.scalar.activation(s_[:], r_[:], AF.Sin, scale=PI)
                if fold:
                    eD.tensor_add(d_[:], s_[:], misc[:, MW_AW:MW_AW + W])
                else:
                    t_ = epool.tile([128, W], F32, name=f"t{nm}")
                    eD.tensor_mul(t_[:], s_[:], misc[:, MW_RW:MW_RW + W])
                    eD.tensor_add(d_[:], t_[:], misc[:, MW_AW:MW_AW + W])
                d4 = d_.rearrange("p (u q) -> p q u", q=4)

                def zk(k):
                    return z_[:, 2 * w * k:2 * w * (k + 1)]

                eP1.tensor_mul(v_[:], d4[:, 2, :], d4[:, 3, :])   # v = d2 d3
                eP0.tensor_mul(zk(1), d4[:, 0, :], d4[:, 1, :])   # z1 = d0 d1
                eP1.tensor_mul(zk(0), d4[:, 1, :], v_[:])         # z0 = d1 v
                eP0.tensor_mul(zk(2), zk(1), d4[:, 2, :])         # z2 = z1 d2
                eP1.tensor_mul(zk(3), zk(1), v_[:])               # z3 = z1 v

                for c in range(2):
                    eZ.tensor_mul(Zw[:, c, :], z_[:], wcs_blk(nm, c, W))
                co = _COFF[nm]
                Ot, cb = (O2a, co) if co < O1C else (O2b, co - O1C)
                for c in range(2):
                    red = Zw[:, c, :].rearrange("p (k g r) -> p g k r",
                                                k=4, r=2)        # [p, w, 4, 2]
                    nc.vector.tensor_reduce(Ot[:, cb + c * w:cb + (c + 1) * w],
                                            red, AX.XY, AL.add)
                    if abs(bc2[c]) > 1e-30:
                        nc.vector.tensor_scalar_add(
                            Ot[:, cb + c * w:cb + (c + 1) * w],
                            Ot[:, cb + c * w:cb + (c + 1) * w], float(bc2[c]))

            # steady chunks: matmuls + full epilogue as each completes
            for (nm, g0, w, wa, mode) in CHUNKS[:-2]:
                E = emit_mms(nm, g0, w, wa)
                emit_chain(nm, w, mode, emit_psum_reads(nm, w, E))
            # two tail chunks: PSUM reads interleaved first, then
            # parallel single-engine chains (G then V)
            tails = CHUNKS[-2:]
            Es = [emit_mms(nm, g0, w, wa) for (nm, g0, w, wa, mode) in tails]
            rs = [emit_psum_reads(nm, w, E)
                  for (nm, g0, w, wa, mode), E in zip(tails, Es)]
            for (nm, g0, w, wa, mode), r_ in zip(tails, rs):
                emit_chain(nm, w, mode, r_)

            # stores: big column block behind all x on sync-q; tail block
            # PE-transposed so the last store is O2C row-packets
            nc.sync.dma_start(o1_d[:], O2a[:])
            pT = pspoolT.tile([128, 128], F32, name="pT")
            nc.tensor.transpose(pT[0:O2C, 0:128], O2b[:], ident[:])
            oT = epool.tile([O2C, 128], F32, name="oT")
            nc.vector.tensor_copy(oT[:], pT[0:O2C, 0:128])
            nc.scalar.dma_start(o2_d[:], oT[:])

    return nc


_NC_CACHE = {}


def _get_nc(consts, split=True):
    key = ("nc5", split, consts)
    if key not in _NC_CACHE:
        nc = _build_nc(consts)
        if split:
            _split_waits(nc)
        _NC_CACHE[key] = nc
    return _NC_CACHE[key]


def _qubit_abc(q_params):
    """Exact (a_i, b_i, c_i) with d_i(theta) = a + b sin(theta) + c cos(theta)."""
    out = np.zeros((NQ, 3), np.float64)
    for i in range(NQ):
        pa, pb, pc = [float(v) for v in q_params[3 * i:3 * i + 3]]

        def rx(t):
            return np.array([[np.cos(t / 2), -1j * np.sin(t / 2)],
                             [-1j * np.sin(t / 2), np.cos(t / 2)]])

        def ry(t):
            return np.array([[np.cos(t / 2), -np.sin(t / 2)],
                             [np.sin(t / 2), np.cos(t / 2)]])

        def rz(t):
            return np.array([[np.exp(-0.5j * t), 0], [0, np.exp(0.5j * t)]])

        H = np.array([[1, 1], [1, -1]]) / np.sqrt(2)
        U = rz(pc) @ ry(pb) @ rx(pa)

        def dfun(theta):
            v = U @ ry(theta) @ H @ np.array([1.0, 0.0])
            pr = np.abs(v) ** 2
            return pr[0] - pr[1]

        d0, dpi, dh = dfun(0.0), dfun(np.pi), dfun(np.pi / 2)
        a = (d0 + dpi) / 2
        c = (d0 - dpi) / 2
        b = dh - a
        out[i] = (a, b, c)
    return out


def _make_consts(b_ctq, q_params, W_cls, b_cls):
    abc = _qubit_abc(q_params)
    R4, a4, bs = np.zeros(4), np.zeros(4), np.zeros(8)
    for i in range(4):
        a, b, c_ = abc[i]
        R4[i] = np.hypot(b, c_)
        a4[i] = a
    for j in range(8):
        _, b, c_ = abc[j % 4]
        bs[j] = b_ctq[j] + np.arctan2(c_, b) / np.pi
    fold = bool(np.min(R4) > 1e-3)
    bc2 = tuple(float(np.float32(v)) for v in b_cls)
    consts = (bc2, fold)

    # folded per-qubit offset and per-k product of R factors
    if fold:
        ap = a4 / R4
        RP = np.array([R4[1] * R4[2] * R4[3], R4[0] * R4[1],
                       R4[0] * R4[1] * R4[2], R4[0] * R4[1] * R4[2] * R4[3]])
    else:
        ap = a4
        RP = np.ones(4)

    misc = np.zeros((128, MW1), np.float16)
    misc[:, MW_ONES:MW_ONES + 128] = 1.0
    misc[:, MW_AW:MW_RW] = np.tile(ap, CBW // 4).astype(np.float16)
    misc[:, MW_RW:MW1] = np.tile(R4 if not fold else np.ones(4),
                                 CBW // 4).astype(np.float16)
    # bias rows: row0 = fp16 hi, row1 = residual lo (hi+lo == fp32 bs)
    bias2 = np.zeros((2, CBW), np.float16)
    bs_t = np.tile(bs, CBW // 8)
    bhi = bs_t.astype(np.float16)
    bias2[0, :] = bhi
    bias2[1, :] = (bs_t - bhi.astype(np.float64)).astype(np.float16)

    # wcs block per chunk: [2 c][4 k][2w (g,r)] = 0.5 W_cls[c,k] RP[k]
    wcs = np.zeros((128, WCW), np.float16)
    wp = 0.5 * np.asarray(W_cls, np.float64)
    for (nm, g0, w, wa, mode) in CHUNKS:
        wo = _WOFF[nm]
        for c in range(2):
            for k in range(4):
                lo = wo + c * 8 * w + k * 2 * w
                wcs[:, lo:lo + 2 * w] = np.float16(wp[c, k] * RP[k])
    return consts, misc, bias2, wcs


def make_in_maps(x, W_ctq, b_ctq, q_params, W_cls, b_cls):
    consts, misc, bias2, wcs = _make_consts(np.asarray(b_ctq, np.float32),
                                     np.asarray(q_params, np.float32),
                                     np.asarray(W_cls, np.float32),
                                     np.asarray(b_cls, np.float32))
    wt = np.asarray(W_ctq, np.float32).T                        # [512, 8]
    misc[:, MW_WFA:MW_WFA + 32] = \
        wt.reshape(NCH, 128, 8).transpose(1, 0, 2).reshape(128, 32)
    misc = np.ascontiguousarray(misc)
    ident = np.eye(128, dtype=np.float32)
    x = np.asarray(x, np.float32)
    in_maps = []
    for c in range(NCORES):
        xs = x[c * BC:(c + 1) * BC]                             # [8192, 512]
        # relayout: [p, g*512 + k*128 + ms] = xs[128 g + ms, 128 k + p]
        xa = np.ascontiguousarray(
            xs.reshape(NG, 128, NCH, 128).transpose(3, 0, 2, 1)
              .reshape(128, BC * NCH)).astype(np.float16)
        in_maps.append({"xa": xa, "misc": misc, "bias2": bias2,
                        "wcs": wcs, "ident": ident})
    return in_maps, consts


def assemble_output(results):
    out = np.empty((B, 2), np.float32)
    for core in range(NCORES):
        o1 = results[core]["o1"]                                 # [128, O1C]
        o2 = results[core]["o2"]                                 # [O2C, 128]
        for (nm, g0, w, wa, mode) in CHUNKS:
            co = _COFF[nm]
            for c in range(2):
                if co < O1C:
                    blk = o1[:, co + c * w:co + (c + 1) * w]     # [128, w]
                else:
                    blk = o2[co - O1C + c * w:co - O1C + (c + 1) * w, :].T
                # blk[p, g] = out_c(sample 128 (g0+g) + p)
                out[core * BC + 128 * g0:core * BC + 128 * (g0 + w), c] = \
                    blk.T.reshape(-1)
    return out


def kernel(x, W_ctq, b_ctq, q_params, W_cls, b_cls):
    in_maps, consts = make_in_maps(x, W_ctq, b_ctq, q_params, W_cls, b_cls)
    nc = _get_nc(consts)
    res = bass_utils.run_bass_kernel_spmd(nc, in_maps, core_ids=list(range(NCORES)))
    return assemble_output(res.results)


# revision 50
# speedup vs baseline: 1.3285x; 1.1882x over previous
"""Trainium2 Bass kernel for nn_BinaryQuantumClassifier.

Math: the 4-qubit circuit collapses to a closed form. Per sample, with
theta_j = pi * (x @ W_ctq.T + b_ctq)_j  (j = 4r + i, reuse r, qubit i):
    d_i = a_i + R_i sin(pi * E_j),   E_j = (x @ W_ctq.T)_j + bs_j
and the CNOT chain maps Z-expectations to products of the d_i:
    z0 = d1 d2 d3, z1 = d0 d1, z2 = d0 d1 d2, z3 = d0 d1 d2 d3.
With d'_i = sin(pi E) + a_i/R_i the R_i factors fold into the final
class weights per k (wcs), so out = (mean_r z) @ W_cls.T + b_cls.

Device plan per core (8192 samples = 64 groups of 128). HBM-bound
streaming x fp16 (~390 GB/s aggregate over both HWDGE queues, which
drain CONCURRENTLY at ~half rate each). Findings baked in from five
measured iterations:
  - 6 chunks [20, 16, 12, 8, 5, 3] groups; each chunk is one half-tile
    on EACH queue so it completes at the cumulative-bytes point of the
    combined stream (per-queue-contiguous tiles complete at 2x
    cumulative time and dump half the work past the stream end).
  - x is the PE's STATIONARY operand (lhsT [128 D x 128 samples]), rhs
    the fp16 W chunk [128 D x 8]; per-chunk phase-shift bias via one
    K=2 matmul of fp16 hi/lo rows (fp32-exact), accumulated in PSUM.
    No other PE work mid-stream.
  - Groups arrive LINEARLY in time, so epilogue throughput must beat
    ~3 groups/us; DVE/GpSimd ops cost 130-250ns FIXED each (GpSimd
    runs at 0.42 efficiency), which measured out as the second wall.
    v7 therefore splits the epilogue:
      per chunk (cheap, latency-tolerant):
        t1 = Identity(E + 1.5*2^24); k2 = Identity(t1 - 1.5*2^24)
        (ScalarE ACTs - the fp32 round-to-even trick; ACT reads PSUM)
        r = E - k2 on Vector (the only per-chunk DVE op)
        s = Sin(pi r) = sin(pi E) (ScalarE) into the batch s-buffer
      per BATCH of two chunks (halves the fixed-op count):
        d' = s + aw'; v = d2 d3; z1 = d0 d1; z0 = d1 v; z2 = z1 d2;
        z3 = z1 v (fp16); Zw_c = z wcs_c (fp16, x2); out_c =
        reduce_{k,r} Zw_c (Vector XY, x2) -> O2 column block
    Batches 0/1 split ops V/G; the tail batch runs all-Vector with its
    two chunks' r ops interleaved ahead of it.
  - wcs const blocks ride the queues right AFTER the first half-tiles;
    misc/bias2/cf lead queue A (needed by the first bias matmul), f32
    identity trails queue B.
  - Stores: batches 0/1 share one [128, 112] column store whose
    descriptor sits behind all x on the sync queue (transfer overlaps
    the tail); the 16-col tail batch is PE-transposed to [16, 128] so
    the final store is 16 row-packets instead of 128 column-packets.
b_cls generality: nonzero bias folds in as 2 tensor_scalar_add per
batch (the graded b_cls == 0 path emits nothing); degenerate R_i ~ 0
switches to an unfolded build (extra batch multiply) via the consts
cache key.
"""

import numpy as np

import concourse.bass as bass
import concourse.mybir as mybir
from concourse import bass_utils
from concourse.tile import TileContext

B, D, NQ = 65536, 512, 4
NCORES = 8
BC = B // NCORES            # 8192 samples per core
NCH = D // 128              # 4 K-chunks
NG = BC // 128              # 64 sample-groups per core (128 samples each)
GW = NCH * 128              # 512: x columns per sample-group

# chunks: (name, group_start, n_groups, groups_on_queue_A)
CHUNKS = [
    ("c0", 0, 20, 10), ("c1", 20, 16, 8),
    ("c2", 36, 12, 6), ("c3", 48, 8, 4),
    ("c4", 56, 5, 3), ("c5", 61, 3, 2),
]
# batches of two chunks; mode 'vg' splits ops V/G, 'v' = all-Vector
BATCHES = [(0, 1, "vg"), (2, 3, "vg"), (4, 5, "v")]
N_O1 = 2                    # first N_O1 batches -> column store o1
WMAX = max(w for (_n, _g, w, _a) in CHUNKS)          # 20
CBW = 8 * WMAX                                       # 160: chunk block width
GBMAX = max(CHUNKS[i][2] + CHUNKS[j][2] for (i, j, _m) in BATCHES)
BBW = 8 * GBMAX                                      # 288: batch block width
PI = float(np.pi)
M2 = float(np.float32(1.5 * 2 ** 24))   # round-to-even-integer magic
MM_DT = mybir.dt.float16    # PE operand / const dtype
F32 = mybir.dt.float32
AL = mybir.AluOpType
AF = mybir.ActivationFunctionType
AX = mybir.AxisListType
# misc (queue A, fp16): wfa 32 | ones 128 | aw BBW | Rw BBW
MW_WFA, MW_ONES, MW_AW = 0, 32, 160
MW_RW = MW_AW + BBW
MW1 = MW_RW + BBW

# batch group ranges, O2 column offsets, wcs offsets
_BG = []                                # (g0, gb) per batch
_WOFF = []
_wc = 0
for (i, j, _m) in BATCHES:
    g0 = CHUNKS[i][1]
    gb = CHUNKS[i][2] + CHUNKS[j][2]
    _BG.append((g0, gb))
    _WOFF.append(_wc)
    _wc += 16 * gb
WCW = _wc                               # 1024
OCOLS = 2 * NG                          # 128
O1C = 2 * (_BG[N_O1][0])                # 112 cols -> o1
O2C = OCOLS - O1C                       # 16 cols -> transposed o2
WCA = _WOFF[1]                          # batch0 wcs -> queue A, rest -> B


def _split_waits(nc, max_waits=1):
    """walrus in this env accepts at most one sync-wait per instruction;
    move extras onto preceding same-engine NoOps."""
    for fn in nc.m.functions:
        for blk in fn.blocks:
            new_list = []
            for inst in blk.instructions:
                si = inst.sync_info
                if si is not None and len(si.on_wait) > max_waits:
                    waits = list(si.on_wait)
                    keep, extra = waits[-max_waits:], waits[:-max_waits]
                    for k, w in enumerate(extra):
                        new_list.append(mybir.InstNoOp(
                            name=f"{inst.name}-ws{k}", engine=inst.engine,
                            ins=[], outs=[],
                            sync_info=mybir.SyncInfo(on_wait=[w], on_update=[])))
                    si.on_wait = keep
                    inst.sync_info = si
                new_list.append(inst)
            blk.instructions = new_list


def _build_nc(consts):
    """consts: (bc2, fold) immediates; misc/bias2/wcs/cf carry the rest."""
    bc2, fold = consts
    nc = bass.Bass("TRN2", target_bir_lowering=False)
    # x relayout: xa[p, g*512 + k*128 + ms] = x_core[128 g + ms, 128 k + p]
    xa_d = nc.dram_tensor("xa", [128, BC * NCH], MM_DT, kind="ExternalInput").ap()
    misc_d = nc.dram_tensor("misc", [128, MW1], MM_DT, kind="ExternalInput").ap()
    bias_d = nc.dram_tensor("bias2", [2, CBW], MM_DT, kind="ExternalInput").ap()
    cf_d = nc.dram_tensor("cf", [128, 4], F32, kind="ExternalInput").ap()
    wcs_d = nc.dram_tensor("wcs", [128, WCW], MM_DT, kind="ExternalInput").ap()
    id_d = nc.dram_tensor("ident", [128, 128], F32, kind="ExternalInput").ap()
    o1_d = nc.dram_tensor("o1", [128, O1C], F32, kind="ExternalOutput").ap()
    o2_d = nc.dram_tensor("o2", [O2C, 128], F32, kind="ExternalOutput").ap()

    with TileContext(nc) as tc:
        with tc.tile_pool(name="wp", bufs=1) as wpool, \
             tc.tile_pool(name="xp", bufs=1) as xpool, \
             tc.tile_pool(name="pe", bufs=3, space="PSUM") as pspoolE, \
             tc.tile_pool(name="pt", bufs=1, space="PSUM") as pspoolT, \
             tc.tile_pool(name="ep", bufs=1) as epool:
            xts = {}

            def xtrig(nm, q, g0, w):
                if w == 0:
                    return
                eng = nc.sync if q == 0 else nc.scalar
                xt = xpool.tile([128, w * GW], MM_DT, name=f"x{nm}q{q}")
                eng.dma_start(xt[:], xa_d[:, g0 * GW:(g0 + w) * GW])
                xts[(nm, q)] = xt

            # queue A: cf+misc+bias lead (first bias matmul needs them),
            # then c0's half with batch0's wcs behind; queue B: c0 half,
            # then wcs for batches 1-2, remaining halves, identity last.
            cf = wpool.tile([128, 4], F32, name="cf")
            nc.sync.dma_start(cf[:], cf_d[:])
            misc = wpool.tile([128, MW1], MM_DT, name="misc")
            nc.sync.dma_start(misc[:], misc_d[:])
            bias2 = wpool.tile([2, CBW], MM_DT, name="bias2")
            nc.sync.dma_start(bias2[:], bias_d[:])
            (nm0, g00, w0, wa0) = CHUNKS[0]
            xtrig(nm0, 0, g00, wa0)
            xtrig(nm0, 1, g00 + wa0, w0 - wa0)
            wcsA = wpool.tile([128, WCA], MM_DT, name="wcsA")
            nc.sync.dma_start(wcsA[:], wcs_d[:, 0:WCA])
            wcsB = wpool.tile([128, WCW - WCA], MM_DT, name="wcsB")
            nc.scalar.dma_start(wcsB[:], wcs_d[:, WCA:WCW])
            for (nm, g0, w, wa) in CHUNKS[1:]:
                xtrig(nm, 0, g0, wa)
                xtrig(nm, 1, g0 + wa, w - wa)
            ident = wpool.tile([128, 128], F32, name="ident")
            nc.scalar.dma_start(ident[:], id_d[:])

            def wcs_blk(b, c, W):
                off = _WOFF[b]
                if off < WCA:
                    return wcsA[:, off + c * W:off + (c + 1) * W]
                off -= WCA
                return wcsB[:, off + c * W:off + (c + 1) * W]

            ones = misc[0:2, MW_ONES:MW_ONES + 128]
            O2a = epool.tile([128, O1C], F32, name="O2a")
            O2b = epool.tile([128, O2C], F32, name="O2b")
            sbufs = [epool.tile([128, 8 * gb], F32, name=f"sb{b}")
                     for b, (g0, gb) in enumerate(_BG)]

            def emit_mms(nm, g0, w, wa):
                W = 8 * w
                E = pspoolE.tile([128, CBW], F32, tag="E", name=f"E{nm}")
                nc.tensor.matmul(E[:, 0:W], ones, bias2[:, 0:W],
                                 start=True, stop=False, skip_group_check=True)
                for g in range(w):
                    q = 0 if g < wa else 1
                    xt = xts[(nm, q)]
                    gl = g if g < wa else g - wa
                    for k in range(NCH):
                        off = gl * GW + k * 128
                        nc.tensor.matmul(E[:, 8 * g:8 * g + 8],
                                         xt[:, off:off + 128],
                                         misc[:, MW_WFA + 8 * k:MW_WFA + 8 * k + 8],
                                         start=False, stop=(k == NCH - 1),
                                         skip_group_check=True)
                return E[:, 0:W]

            def emit_chunk_epi(b, nm, g0, w, E):
                """round-to-even on ScalarE (Identity ACTs), r on Vector,
                Sin into the batch s-buffer slice."""
                W = 8 * w
                t1 = epool.tile([128, W], F32, name=f"t1{nm}")
                k2 = epool.tile([128, W], F32, name=f"k2{nm}")
                r_ = epool.tile([128, W], F32, name=f"r{nm}")
                nc.scalar.activation(t1[:], E[:], AF.Identity, bias=cf[:, 0:1])
                nc.scalar.activation(k2[:], t1[:], AF.Identity, bias=cf[:, 1:2])
                nc.vector.tensor_sub(r_[:], E[:], k2[:])
                so = 8 * (g0 - _BG[b][0])
                nc.scalar.activation(sbufs[b][:, so:so + W], r_[:],
                                     AF.Sin, scale=PI)

            def emit_batch(b, mode):
                g0, gb = _BG[b]
                W = 8 * gb
                if mode == "v":
                    eD = eP0 = eP1 = eZ0 = eZ1 = nc.vector
                else:
                    eD, eP0, eP1 = nc.gpsimd, nc.vector, nc.gpsimd
                    eZ0, eZ1 = nc.vector, nc.gpsimd
                s_ = sbufs[b]
                d_ = epool.tile([128, W], MM_DT, name=f"d{b}")
                v_ = epool.tile([128, 2 * gb], MM_DT, name=f"v{b}")
                z_ = epool.tile([128, W], MM_DT, name=f"z{b}")
                Zw = epool.tile([128, 2, W], MM_DT, name=f"Zw{b}")

                if fold:
                    eD.tensor_add(d_[:], s_[:], misc[:, MW_AW:MW_AW + W])
                else:
                    t_ = epool.tile([128, W], F32, name=f"t{b}")
                    eD.tensor_mul(t_[:], s_[:], misc[:, MW_RW:MW_RW + W])
                    eD.tensor_add(d_[:], t_[:], misc[:, MW_AW:MW_AW + W])
                d4 = d_.rearrange("p (u q) -> p q u", q=4)

                def zk(k):
                    return z_[:, 2 * gb * k:2 * gb * (k + 1)]

                eP1.tensor_mul(v_[:], d4[:, 2, :], d4[:, 3, :])   # v = d2 d3
                eP0.tensor_mul(zk(1), d4[:, 0, :], d4[:, 1, :])   # z1 = d0 d1
                eP1.tensor_mul(zk(0), d4[:, 1, :], v_[:])         # z0 = d1 v
                eP0.tensor_mul(zk(2), zk(1), d4[:, 2, :])         # z2 = z1 d2
                eP1.tensor_mul(zk(3), zk(1), v_[:])               # z3 = z1 v

                for c, eZ in ((0, eZ0), (1, eZ1)):
                    eZ.tensor_mul(Zw[:, c, :], z_[:], wcs_blk(b, c, W))
                co = 2 * g0
                Ot, cb = (O2a, co) if co < O1C else (O2b, co - O1C)
                for c in range(2):
                    red = Zw[:, c, :].rearrange("p (k g r) -> p g k r",
                                                k=4, r=2)        # [p, gb, 4, 2]
                    nc.vector.tensor_reduce(Ot[:, cb + c * gb:cb + (c + 1) * gb],
                                            red, AX.XY, AL.add)
                    if abs(bc2[c]) > 1e-30:
                        nc.vector.tensor_scalar_add(
                            Ot[:, cb + c * gb:cb + (c + 1) * gb],
                            Ot[:, cb + c * gb:cb + (c + 1) * gb], float(bc2[c]))

            for b, (i, j, mode) in enumerate(BATCHES):
                for ci in (i, j):
                    (nm, g0, w, wa) = CHUNKS[ci]
                    E = emit_mms(nm, g0, w, wa)
                    emit_chunk_epi(b, nm, g0, w, E)
                emit_batch(b, mode)
                if b == N_O1 - 1:
                    nc.sync.dma_start(o1_d[:], O2a[:])

            # tail store: PE transpose so it is O2C row-packets
            pT = pspoolT.tile([128, 128], F32, name="pT")
            nc.tensor.transpose(pT[0:O2C, 0:128], O2b[:], ident[:])
            oT = epool.tile([O2C, 128], F32, name="oT")
            nc.vector.tensor_copy(oT[:], pT[0:O2C, 0:128])
            nc.scalar.dma_start(o2_d[:], oT[:])

    return nc


_NC_CACHE = {}


def _get_nc(consts, split=True):
    key = ("nc7", split, consts)
    if key not in _NC_CACHE:
        nc = _build_nc(consts)
        if split:
            _split_waits(nc)
        _NC_CACHE[key] = nc
    return _NC_CACHE[key]


def _qubit_abc(q_params):
    """Exact (a_i, b_i, c_i) with d_i(theta) = a + b sin(theta) + c cos(theta)."""
    out = np.zeros((NQ, 3), np.float64)
    for i in range(NQ):
        pa, pb, pc = [float(v) for v in q_params[3 * i:3 * i + 3]]

        def rx(t):
            return np.array([[np.cos(t / 2), -1j * np.sin(t / 2)],
                             [-1j * np.sin(t / 2), np.cos(t / 2)]])

        def ry(t):
            return np.array([[np.cos(t / 2), -np.sin(t / 2)],
                             [np.sin(t / 2), np.cos(t / 2)]])

        def rz(t):
            return np.array([[np.exp(-0.5j * t), 0], [0, np.exp(0.5j * t)]])

        H = np.array([[1, 1], [1, -1]]) / np.sqrt(2)
        U = rz(pc) @ ry(pb) @ rx(pa)

        def dfun(theta):
            v = U @ ry(theta) @ H @ np.array([1.0, 0.0])
            pr = np.abs(v) ** 2
            return pr[0] - pr[1]

        d0, dpi, dh = dfun(0.0), dfun(np.pi), dfun(np.pi / 2)
        a = (d0 + dpi) / 2
        c = (d0 - dpi) / 2
        b = dh - a
        out[i] = (a, b, c)
    return out


def _make_consts(b_ctq, q_params, W_cls, b_cls):
    abc = _qubit_abc(q_params)
    R4, a4, bs = np.zeros(4), np.zeros(4), np.zeros(8)
    for i in range(4):
        a, b, c_ = abc[i]
        R4[i] = np.hypot(b, c_)
        a4[i] = a
    for j in range(8):
        _, b, c_ = abc[j % 4]
        bs[j] = b_ctq[j] + np.arctan2(c_, b) / np.pi
    fold = bool(np.min(R4) > 1e-3)
    bc2 = tuple(float(np.float32(v)) for v in b_cls)
    consts = (bc2, fold)

    if fold:
        ap = a4 / R4
        RP = np.array([R4[1] * R4[2] * R4[3], R4[0] * R4[1],
                       R4[0] * R4[1] * R4[2], R4[0] * R4[1] * R4[2] * R4[3]])
    else:
        ap = a4
        RP = np.ones(4)

    misc = np.zeros((128, MW1), np.float16)
    misc[:, MW_ONES:MW_ONES + 128] = 1.0
    misc[:, MW_AW:MW_RW] = np.tile(ap, BBW // 4).astype(np.float16)
    misc[:, MW_RW:MW1] = np.tile(R4 if not fold else np.ones(4),
                                 BBW // 4).astype(np.float16)
    # bias rows: row0 = fp16 hi, row1 = residual lo (hi+lo == fp32 bs)
    bias2 = np.zeros((2, CBW), np.float16)
    bs_t = np.tile(bs, CBW // 8)
    bhi = bs_t.astype(np.float16)
    bias2[0, :] = bhi
    bias2[1, :] = (bs_t - bhi.astype(np.float64)).astype(np.float16)
    cf = np.zeros((128, 4), np.float32)
    cf[:, 0] = M2
    cf[:, 1] = -M2

    # wcs block per batch: [2 c][4 k][2gb (g,r)] = 0.5 W_cls[c,k] RP[k]
    wcs = np.zeros((128, WCW), np.float16)
    wp = 0.5 * np.asarray(W_cls, np.float64)
    for b, (g0, gb) in enumerate(_BG):
        wo = _WOFF[b]
        for c in range(2):
            for k in range(4):
                lo = wo + c * 8 * gb + k * 2 * gb
                wcs[:, lo:lo + 2 * gb] = np.float16(wp[c, k] * RP[k])
    return consts, misc, bias2, cf, wcs


def make_in_maps(x, W_ctq, b_ctq, q_params, W_cls, b_cls):
    consts, misc, bias2, cf, wcs = _make_consts(
        np.asarray(b_ctq, np.float32), np.asarray(q_params, np.float32),
        np.asarray(W_cls, np.float32), np.asarray(b_cls, np.float32))
    wt = np.asarray(W_ctq, np.float32).T                        # [512, 8]
    misc[:, MW_WFA:MW_WFA + 32] = \
        wt.reshape(NCH, 128, 8).transpose(1, 0, 2).reshape(128, 32)
    misc = np.ascontiguousarray(misc)
    ident = np.eye(128, dtype=np.float32)
    x = np.asarray(x, np.float32)
    in_maps = []
    for c in range(NCORES):
        xs = x[c * BC:(c + 1) * BC]                             # [8192, 512]
        # relayout: [p, g*512 + k*128 + ms] = xs[128 g + ms, 128 k + p]
        xa = np.ascontiguousarray(
            xs.reshape(NG, 128, NCH, 128).transpose(3, 0, 2, 1)
              .reshape(128, BC * NCH)).astype(np.float16)
        in_maps.append({"xa": xa, "misc": misc, "bias2": bias2, "cf": cf,
                        "wcs": wcs, "ident": ident})
    return in_maps, consts


def assemble_output(results):
    out = np.empty((B, 2), np.float32)
    for core in range(NCORES):
        o1 = results[core]["o1"]                                 # [128, O1C]
        o2 = results[core]["o2"]                                 # [O2C, 128]
        for b, (g0, gb) in enumerate(_BG):
            co = 2 * g0
            for c in range(2):
                if co < O1C:
                    blk = o1[:, co + c * gb:co + (c + 1) * gb]   # [128, gb]
                else:
                    blk = o2[co - O1C + c * gb:co - O1C + (c + 1) * gb, :].T
                # blk[p, g] = out_c(sample 128 (g0+g) + p)
                out[core * BC + 128 * g0:core * BC + 128 * (g0 + gb), c] = \
                    blk.T.reshape(-1)
    return out


def kernel(x, W_ctq, b_ctq, q_params, W_cls, b_cls):
    in_maps, consts = make_in_maps(x, W_ctq, b_ctq, q_params, W_cls, b_cls)
    nc = _get_nc(consts)
    res = bass_utils.run_bass_kernel_spmd(nc, in_maps, core_ids=list(range(NCORES)))
    return assemble_output(res.results)
